# revision 19
# baseline (speedup 1.0000x reference)
"""Banded additive attention (width-128) on 8 TRN2 NeuronCores — raw Bass.

Problem: B=2, L=2048, F=128, U=32, WIDTH=128
  q = x@Wt + bh, k = x@Wx                       (host, like baseline)
  s_ij = Wa . tanh(q_i + k_j)                   (j in [i-64, i+63])
  e_ij = exp(sigmoid(s_ij)) * band * mask
  v_i  = sum_j e_ij x_j / (sum_j e_ij + 1e-7)

Sharding: core c handles batch c//4, queries [(c%4)*512, +512).  No
collectives.  Raw Bass; standalone wait_ge with hand-computed thresholds.

v4 pipeline (query-chunked; chunk widths SIZES, all mult of 16):
  DVE  : add   arg[p=(dm,u), (dg,il)] = q4[p, il] + K4[p, 4*dg + il]
         (3 arg slots so adds run 2 chunks ahead)
  ACT  : tanh  in-place on arg (the bulk: 512*128*32 elems/core)
  PE   : 32 accumulating matmuls per chunk into psS[c%3] psum bank:
         mm(ds,g): lhsT = W32g[:, 32g:+32] (block-diag Wa), rhs = dg-block,
         out rows 32ds..+32 (tile_position col = 32ds), d = 32ds+4g+dm
  ACT  : sig = Sigmoid(psum + ba); e = Exp(sig)
  PE   : 2-level shear (radix 8x16): rotation matmuls on col classes with
         compacted outputs (digits recombine to natural order):
         L-lo: rot lo=il%8 on stride-8 classes, e -> psA
         L-hi: rot 8*((c0/8 + hi)%16) on stride-(W/8) classes, M0 -> psB
         (last chunk W=32: single level, 32 one-col rotation matmuls)
  DVE  : M0 = copy(psA);  El = psB * lower-tri mask
  Pool : Eh = psB * upper-tri mask  (parallel with El)
  PE   : v = El.T @ X[t] + Eh.T @ X[t+1]  (X has validity col -> denom)
  DVE  : ov = vp;  DMA out.  Host divides by denom and applies mask.
"""

import numpy as np
import ml_dtypes

B, L, F, U = 2, 2048, 128, 32
WIDTH = 128
EPS = 1e-7
NCORES = 8
QPC = (B * L) // NCORES          # 512 queries per core
NKEY = QPC + WIDTH               # 640 key rows per core
NK4 = NKEY + 4                   # K4 row pitch
BF16 = ml_dtypes.bfloat16

SIZES = [48, 128, 128, 128, 64, 16]      # query chunks (mult of 16, <=128)
C0S = [0, 48, 176, 304, 432, 496]
NCH = len(SIZES)
LAST = NCH - 1                           # direct-shear chunk
# core rotations (2-level shear) + direct rotations for the last chunk
ROTS = [0, 1, 2, 3, 4, 5, 6, 7] + [8 * k for k in range(1, 16)]
NRC = len(ROTS)                          # 23 core matrices
DROTS = [(C0S[LAST] + il) % 128 for il in range(SIZES[LAST])]
for r in DROTS:
    if r not in ROTS:
        ROTS.append(r)
RIDX = {r: i for i, r in enumerate(ROTS)}
NR = len(ROTS)
AMAX = max(SIZES) * U            # 4608: arg buffer cols

# (block, global lo, global hi) pieces per chunk; block t = cols [128t,+128)
PIECES = []
for c in range(NCH):
    lo, hi = C0S[c], C0S[c] + SIZES[c]
    ps = []
    for t in range(4):
        a, b = max(lo, 128 * t), min(hi, 128 * (t + 1))
        if a < b:
            ps.append((t, a, b))
    PIECES.append(ps)
LASTCH = {}                      # chunk completing block t's El/Eh
for c in range(NCH):
    for (t, a, b) in PIECES[c]:
        LASTCH[t] = c
_msk = 0
MSKTHR = {}                      # sMSK threshold per block (2 per piece)
MSKCUM = []                      # cumulative TT count through chunk c
for c in range(NCH):
    for (t, a, b) in PIECES[c]:
        _msk += 2
        MSKTHR[t] = _msk
    MSKCUM.append(_msk)
# sSH count before chunk c's levels: 2 per 2-level chunk, 1 for the last
SHTHR = [2 * c for c in range(NCH)] + [2 * LAST + 1]
assert max(SIZES) <= 128 and all(w % 16 == 0 for w in SIZES)

_built = None


def _build():
    import concourse.bass as bass
    import concourse.mybir as mybir

    f32 = mybir.dt.float32
    bf16 = mybir.dt.bfloat16
    Sig = mybir.ActivationFunctionType.Sigmoid
    Exp = mybir.ActivationFunctionType.Exp
    Tanh = mybir.ActivationFunctionType.Tanh
    Mult = mybir.AluOpType.mult
    AP = bass.AP

    nc = bass.Bass()

    q4_d = nc.dram_tensor("q4", [128, QPC], bf16, kind="ExternalInput")
    K4_d = nc.dram_tensor("K4", [128, NK4], bf16, kind="ExternalInput")
    W_d = nc.dram_tensor("W32g", [128, 256], bf16, kind="ExternalInput")
    # Rm: NR rotation matrices then maskl/maskh [128, 256]
    R_d = nc.dram_tensor("Rm", [128, 128 * NR + 256], bf16,
                         kind="ExternalInput")
    Xe_d = nc.dram_tensor("Xe", [NKEY, 132], bf16, kind="ExternalInput")
    ba_d = nc.dram_tensor("bat", [128, 2], f32, kind="ExternalInput")
    out_d = nc.dram_tensor("out", [QPC, 132], f32, kind="ExternalOutput")

    al = nc.alloc_sbuf_tensor
    q4 = al("q4s", [128, QPC], bf16)
    K4 = al("K4s", [128, NK4], bf16)
    W32 = al("W32s", [128, 256], bf16)
    Rm = al("Rms", [128, 128 * NR + 256], bf16)
    X5 = al("X5s", [128, 660], bf16)
    bat = al("bats", [128, 2], f32)
    arg = [al(f"arg{i}", [128, AMAX], bf16) for i in range(3)]
    sig = al("sigs", [128, QPC], bf16)
    ee = al("ees", [128, QPC], bf16)
    M0 = al("M0s", [128, QPC], bf16)
    El = [al(f"el{t}", [128, 128], bf16) for t in range(4)]
    Eh = [al(f"eh{t}", [128, 128], bf16) for t in range(4)]
    ov = [al(f"ov{t}", [128, 132], f32) for t in range(4)]
    MKL = 128 * NRC              # maskl col offset in Rm
    MKH = 128 * NRC + 128

    def rcol(r):
        i = RIDX[r]
        return 128 * i if i < NRC else 128 * i + 256

    ap_ = nc.alloc_psum_tensor
    psS = [ap_(f"psS{i}", [128, max(SIZES)], f32) for i in range(3)]
    psA = ap_("psA", [128, QPC], f32)
    psB = ap_("psB", [128, QPC], f32)
    vp = [ap_(f"vp{i}", [128, 132], f32) for i in range(2)]

    sem = nc.alloc_semaphore
    sINA, sINB, sINC, sIND, sINE, sINF, sING, sINH, sINR = (sem(n) for n in
        ("sINA", "sINB", "sINC", "sIND", "sINE", "sINF", "sING", "sINH",
         "sINR"))
    sADD, sTANH, sMM, sSIG, sEXP = (sem(n) for n in
        ("sADD", "sTANH", "sMM", "sSIG", "sEXP"))
    sSH, sSCP, sMSK, sMSKP, sVMM, sEPI, sOUT = (sem(n) for n in
        ("sSH", "sSCP", "sMSK", "sMSKP", "sVMM", "sEPI", "sOUT"))

    W0 = SIZES[0]
    K4P = 4 * 31 + W0 + 16       # first-chunk K4 cols (rounded up)

    with nc.Block() as block:

        @block.sync
        def _(sync):
            # SP queue: q4p, q4r, K4r, bat, Xe, Rm-extra
            sync.dma_start(q4[:, 0:W0], q4_d[:, 0:W0]).then_inc(sINA, 16)
            sync.dma_start(q4[:, W0:QPC], q4_d[:, W0:QPC]).then_inc(sINH, 16)
            sync.dma_start(K4[:, K4P:NK4], K4_d[:, K4P:NK4]).then_inc(sINC, 16)
            sync.dma_start(bat[:, :], ba_d[:, :]).then_inc(sIND, 16)
            sync.dma_start(AP(X5, 0, [[660, 128], [132, 5], [1, 132]]),
                           AP(Xe_d, 0, [[132, 128], [128 * 132, 5], [1, 132]])
                           ).then_inc(sINE, 16)
            sync.dma_start(Rm[:, 128 * NRC + 256:],
                           R_d[:, 128 * NRC + 256:]).then_inc(sINR, 16)
            for t in range(4):
                sync.wait_ge(sEPI, t + 1)
                sync.dma_start(out_d[128 * t:128 * (t + 1), :],
                               ov[t][:, :]).then_inc(sOUT, 16)
            sync.wait_ge(sOUT, 64)

        @block.vector
        def _(vector):
            def add(c):
                c0, W = C0S[c], SIZES[c]
                if c == 0:
                    vector.wait_ge(sINA, 16)
                    vector.wait_ge(sING, 16)
                elif c == 1:
                    vector.wait_ge(sINH, 16)
                    vector.wait_ge(sINC, 16)
                else:
                    vector.wait_ge(sMM, c - 2)       # arg slot c%3 free
                a = arg[c % 3]
                vector.tensor_add(
                    AP(a, 0, [[AMAX, 128], [W, U], [1, W]]),
                    AP(q4, c0, [[QPC, 128], [0, U], [1, W]]),
                    AP(K4, c0, [[NK4, 128], [4, U], [1, W]])
                ).then_inc(sADD, 1)

            def m0copy(c):
                c0, W = C0S[c], SIZES[c]
                vector.wait_ge(sSH, SHTHR[c] + 1)
                vector.tensor_copy(M0[:, c0:c0 + W],
                                   psA[:, c0:c0 + W]).then_inc(sSCP, 1)

            def el(c):
                vector.wait_ge(sSH, SHTHR[c + 1])
                for (t, a, b) in PIECES[c]:
                    la, lb = a - 128 * t, b - 128 * t
                    vector.tensor_tensor(
                        El[t][:, la:lb], psB[:, a:b],
                        Rm[:, MKL + la:MKL + lb], Mult).then_inc(sMSK, 1)
                    vector.tensor_tensor(
                        Eh[t][:, la:lb], psB[:, a:b],
                        Rm[:, MKH + la:MKH + lb], Mult).then_inc(sMSK, 1)

            def epi(t):
                vector.wait_ge(sVMM, t + 1)
                vector.tensor_copy(ov[t][:, :], vp[t % 2][:, :]
                                   ).then_inc(sEPI, 1)

            add(0); add(1); add(2); add(3)
            m0copy(0); el(0)
            add(4)
            m0copy(1); el(1)
            add(5)
            m0copy(2); el(2)
            epi(0)
            m0copy(3); el(3)
            epi(1)
            m0copy(4); el(4)
            el(5)
            epi(2)
            epi(3)

        @block.scalar
        def _(scalar):
            def tanh(c):
                W = SIZES[c]
                scalar.wait_ge(sADD, c + 1)
                a = arg[c % 3]
                scalar.activation(AP(a, 0, [[AMAX, 128], [1, U * W]]),
                                  AP(a, 0, [[AMAX, 128], [1, U * W]]),
                                  Tanh).then_inc(sTANH, 1)

            def sigm(c):
                c0, W = C0S[c], SIZES[c]
                if c == 0:
                    scalar.wait_ge(sIND, 16)
                scalar.wait_ge(sMM, c + 1)
                scalar.activation(sig[:, c0:c0 + W], psS[c % 3][:, 0:W],
                                  Sig, bias=bat[:, 0:1]).then_inc(sSIG, 1)

            def expo(c):
                c0, W = C0S[c], SIZES[c]
                scalar.wait_ge(sSIG, c + 1)
                scalar.activation(ee[:, c0:c0 + W], sig[:, c0:c0 + W],
                                  Exp).then_inc(sEXP, 1)

            # sig/exp software-pipelined so the sSIG wait is pre-satisfied
            tanh(0); tanh(1)
            sigm(0)
            tanh(2)
            for c in range(1, NCH):
                sigm(c)
                expo(c - 1)
                if c + 2 < NCH:
                    tanh(c + 2)
            expo(NCH - 1)

        @block.tensor
        def _(tensor):
            def score(c):
                c0, W = C0S[c], SIZES[c]
                tensor.wait_ge(sTANH, c + 1)
                if c == 0:
                    tensor.wait_ge(sINB, 16)
                if c >= 3:
                    tensor.wait_ge(sSIG, c - 2)      # psS[c%3] free
                a = arg[c % 3]
                for ds in range(4):
                    for g in range(8):
                        mm = tensor.matmul(
                            psS[c % 3][32 * ds:32 * (ds + 1), 0:W],
                            W32[:, 32 * g:32 * (g + 1)],
                            AP(a, (8 * ds + g) * W, [[AMAX, 128], [1, W]]),
                            start=(g == 0), stop=(g == 7),
                            tile_position=(0, 32 * ds))
                        if ds == 3 and g == 7:
                            mm.then_inc(sMM, 1)

            def shear(c):
                c0, W = C0S[c], SIZES[c]
                nlo = W // 8
                # L-lo: rot lo on cols c0+lo::8, e -> psA (compacted)
                tensor.wait_ge(sEXP, c + 1)
                if c == 0:
                    tensor.wait_ge(sINF, 16)
                else:
                    # L-lo group start wipes psA bank: prior m0copy must
                    # be done; same for psB vs prior chunk's El/Eh reads
                    tensor.wait_ge(sSCP, c)
                    tensor.wait_ge(sMSK, MSKCUM[c - 1])
                for lo in range(8):
                    r = rcol(lo)
                    mm = tensor.matmul(
                        psA[:, c0 + lo * nlo:c0 + (lo + 1) * nlo],
                        Rm[:, r:r + 128],
                        AP(ee, c0 + lo, [[QPC, 128], [8, nlo]]),
                        start=(lo == 0), stop=(lo == 7),
                        skip_group_check=True)
                    if lo == 7:
                        mm.then_inc(sSH, 1)
                # L-hi: rot 8*((c0/8 + hi)%16) on stride-nlo classes,
                # M0 -> psB; compaction digits recombine to natural order
                tensor.wait_ge(sSCP, c + 1)
                for hi in range(nlo):
                    r = rcol(8 * ((c0 // 8 + hi) % 16))
                    mm = tensor.matmul(
                        psB[:, c0 + 8 * hi:c0 + 8 * (hi + 1)],
                        Rm[:, r:r + 128],
                        AP(M0, c0 + hi, [[QPC, 128], [nlo, 8]]),
                        start=(hi == 0), stop=(hi == nlo - 1),
                        skip_group_check=True)
                    if hi == nlo - 1:
                        mm.then_inc(sSH, 1)

            def shear_direct(c):
                c0, W = C0S[c], SIZES[c]
                tensor.wait_ge(sEXP, c + 1)
                tensor.wait_ge(sINR, 16)
                tensor.wait_ge(sMSK, MSKCUM[c - 1])
                for il in range(W):
                    r = rcol((c0 + il) % 128)
                    mm = tensor.matmul(
                        psB[:, c0 + il:c0 + il + 1],
                        Rm[:, r:r + 128],
                        ee[:, c0 + il:c0 + il + 1],
                        start=(il == 0), stop=(il == W - 1),
                        skip_group_check=True)
                    if il == W - 1:
                        mm.then_inc(sSH, 1)

            def value(t):
                tensor.wait_ge(sMSK, MSKTHR[t])
                if t == 0:
                    tensor.wait_ge(sINE, 16)
                if t >= 2:
                    tensor.wait_ge(sEPI, t - 1)      # vp slot t%2 free
                tensor.matmul(vp[t % 2][:, :], El[t][:, :],
                              X5[:, 132 * t:132 * t + 132],
                              start=True, stop=False, skip_group_check=True)
                tensor.matmul(vp[t % 2][:, :], Eh[t][:, :],
                              X5[:, 132 * (t + 1):132 * (t + 1) + 132],
                              start=False, stop=True,
                              skip_group_check=True).then_inc(sVMM, 1)

            for c in range(NCH):
                score(c)
                if c >= 1 and c - 1 != LAST:
                    shear(c - 1)
                for t in range(4):
                    if LASTCH[t] == c - 2:
                        value(t)
            shear_direct(LAST)
            for t in range(4):
                if LASTCH[t] >= NCH - 2:
                    value(t)

        @block.gpsimd
        def _(gpsimd):
            gpsimd.dma_start(K4[:, 0:K4P], K4_d[:, 0:K4P]).then_inc(sING, 16)
            gpsimd.dma_start(W32[:, :], W_d[:, :]).then_inc(sINB, 16)
            gpsimd.dma_start(Rm[:, 0:128 * NRC + 256],
                             R_d[:, 0:128 * NRC + 256]).then_inc(sINF, 16)

    nc.finalize()
    return nc


def _prep_inputs(x, mask, Wt, Wx, bh, Wa, ba):
    """Build the 8 per-core input maps (host-side sharding + projections)."""
    x64 = x.astype(np.float64)
    # rotation matrices R_r[(c - r) % 128, c] = 1, then tri masks
    Rm = np.zeros((128, 128 * NR + 256), np.float32)
    cix = np.arange(128)
    for i, r in enumerate(ROTS):
        base = 128 * i if i < NRC else 128 * i + 256
        Rm[(cix - r) % 128, base + cix] = 1.0
    cc = cix[:, None]
    il = cix[None, :]
    Rm[:, 128 * NRC:128 * NRC + 128] = (cc >= il).astype(np.float32)
    Rm[:, 128 * NRC + 128:128 * NRC + 256] = (cc < il).astype(np.float32)
    Rm = Rm.astype(BF16)
    # W32g: mm g maps rows (dm,u) -> col 4g+dm with weight Wa[u]
    W32 = np.zeros((128, 256), np.float32)
    for g in range(8):
        for dm in range(4):
            W32[32 * dm:32 * (dm + 1), 32 * g + 4 * g + dm] = Wa[:, 0]
    W32 = W32.astype(BF16)
    in_maps = []
    for c in range(NCORES):
        b = c // 4
        qs = (c % 4) * QPC
        q = (x64[b] @ Wt.astype(np.float64) + bh.astype(np.float64))
        k = (x64[b] @ Wx.astype(np.float64))
        qT = q[qs:qs + QPC].T.astype(np.float32)          # [32, 512]
        q4 = np.tile(qT, (4, 1)).astype(BF16)             # [128, 512]
        kx = np.zeros((NKEY + 8, U), np.float64)
        lo = qs - 64
        s0, s1 = max(0, lo), min(L, lo + NKEY)
        kx[s0 - lo:s1 - lo] = k[s0:s1]
        K4 = np.zeros((128, NK4), np.float32)
        for dm in range(4):
            K4[32 * dm:32 * (dm + 1), :] = kx[dm:dm + NK4].T
        K4 = K4.astype(BF16)
        Xe = np.zeros((NKEY, 132), np.float32)
        mk = mask[b].astype(np.float32)
        xr = np.zeros((NKEY, F), np.float32)
        xr[s0 - lo:s1 - lo] = x[b, s0:s1] * mk[s0:s1, None]
        Xe[:, :F] = xr
        val = np.zeros(NKEY, np.float32)
        val[s0 - lo:s1 - lo] = mk[s0:s1]
        Xe[:, F] = val
        Xe = Xe.astype(BF16)
        bat = np.zeros((128, 2), np.float32)
        bat[:, 0] = float(ba[0])
        in_maps.append({"q4": q4, "K4": K4, "W32g": W32, "Rm": Rm,
                        "Xe": Xe, "bat": bat})
    return in_maps


def kernel(x, mask, Wt, Wx, bh, Wa, ba, _want_results=False):
    global _built
    from concourse.bass_utils import run_bass_kernel_spmd
    x = np.asarray(x)
    mask = np.asarray(mask)
    Wt, Wx, bh, Wa, ba = (np.asarray(a) for a in (Wt, Wx, bh, Wa, ba))
    if _built is None:
        _built = _build()
    nc = _built
    in_maps = _prep_inputs(x, mask, Wt, Wx, bh, Wa, ba)
    res = run_bass_kernel_spmd(nc, in_maps, core_ids=list(range(NCORES)))
    v = np.zeros((B, L, F), np.float32)
    for c in range(NCORES):
        b = c // 4
        qs = (c % 4) * QPC
        o = res.results[c]["out"]
        v[b, qs:qs + QPC] = o[:, :F] / (o[:, F:F + 1] + EPS)
    v *= mask.astype(np.float32)[:, :, None]
    if _want_results:
        return v, res
    return v


# revision 20
# speedup vs baseline: 1.0901x; 1.0901x over previous
"""Banded additive attention (width-128) on 8 TRN2 NeuronCores — raw Bass.

Problem: B=2, L=2048, F=128, U=32, WIDTH=128
  q = x@Wt + bh, k = x@Wx                       (host, like baseline)
  s_ij = Wa . tanh(q_i + k_j)                   (j in [i-64, i+63])
  e_ij = exp(sigmoid(s_ij)) * band * mask
  v_i  = sum_j e_ij x_j / (sum_j e_ij + 1e-7)

Sharding: core c handles batch c//4, queries [(c%4)*512, +512).  No
collectives.  Raw Bass; standalone wait_ge with hand-computed thresholds.

v4 pipeline (query-chunked; chunk widths SIZES, all mult of 16):
  DVE  : add   arg[p=(dm,u), (dg,il)] = q4[p, il] + K4[p, 4*dg + il]
         (3 arg slots so adds run 2 chunks ahead)
  ACT  : tanh  in-place on arg (the bulk: 512*128*32 elems/core)
  PE   : 32 accumulating matmuls per chunk into psS[c%3] psum bank:
         mm(ds,g): lhsT = W32g[:, 32g:+32] (block-diag Wa), rhs = dg-block,
         out rows 32ds..+32 (tile_position col = 32ds), d = 32ds+4g+dm
  ACT  : sig = Sigmoid(psum + ba); e = Exp(sig)
  PE   : 2-level shear (radix 8x16): rotation matmuls on col classes with
         compacted outputs (digits recombine to natural order):
         L-lo: rot lo=il%8 on stride-8 classes, e -> psA
         L-hi: rot 8*((c0/8 + hi)%16) on stride-(W/8) classes, M0 -> psB
         (last chunk W=32: single level, 32 one-col rotation matmuls)
  DVE  : M0 = copy(psA);  El = psB * lower-tri mask
  Pool : Eh = psB * upper-tri mask  (parallel with El)
  PE   : v = El.T @ X[t] + Eh.T @ X[t+1]  (X has validity col -> denom)
  DVE  : ov = vp;  DMA out.  Host divides by denom and applies mask.
"""

import numpy as np
import ml_dtypes

B, L, F, U = 2, 2048, 128, 32
WIDTH = 128
EPS = 1e-7
NCORES = 8
QPC = (B * L) // NCORES          # 512 queries per core
NKEY = QPC + WIDTH               # 640 key rows per core
NK4 = NKEY + 4                   # K4 row pitch
BF16 = ml_dtypes.bfloat16

SIZES = [80, 112, 128, 128, 48, 16]      # query chunks (mult of 16, <=128)
C0S = [0, 80, 192, 320, 448, 496]
NCH = len(SIZES)
NDIR = 2                                 # last NDIR chunks use direct shear
# core rotations (2-level shear) + direct rotations for the last chunks
ROTS = [0, 1, 2, 3, 4, 5, 6, 7] + [8 * k for k in range(1, 16)]
NRC = len(ROTS)                          # 23 core matrices
DROTS = []
for c in range(NCH - NDIR, NCH):
    DROTS += [(C0S[c] + il) % 128 for il in range(SIZES[c])]
for r in DROTS:
    if r not in ROTS:
        ROTS.append(r)
RIDX = {r: i for i, r in enumerate(ROTS)}
NR = len(ROTS)
AMAX = max(SIZES) * U            # 4608: arg buffer cols

# (block, global lo, global hi) pieces per chunk; block t = cols [128t,+128)
PIECES = []
for c in range(NCH):
    lo, hi = C0S[c], C0S[c] + SIZES[c]
    ps = []
    for t in range(4):
        a, b = max(lo, 128 * t), min(hi, 128 * (t + 1))
        if a < b:
            ps.append((t, a, b))
    PIECES.append(ps)
LASTCH = {}                      # chunk completing block t's El/Eh
for c in range(NCH):
    for (t, a, b) in PIECES[c]:
        LASTCH[t] = c
_msk = 0
MSKTHR = {}                      # sMSK threshold per block (2 per piece)
MSKCUM = []                      # cumulative TT count through chunk c
for c in range(NCH):
    for (t, a, b) in PIECES[c]:
        _msk += 2
        MSKTHR[t] = _msk
    MSKCUM.append(_msk)
# sSH increments: 2 per 2-level chunk, 1 per direct chunk.
# SHTHR[c] = count before chunk c's shear; SHTHR[NCH] = total
SHTHR = [0]
for c in range(NCH):
    SHTHR.append(SHTHR[-1] + (1 if c >= NCH - NDIR else 2))
assert max(SIZES) <= 128 and all(w % 16 == 0 for w in SIZES)

_built = None


def _build():
    import concourse.bass as bass
    import concourse.mybir as mybir

    f32 = mybir.dt.float32
    bf16 = mybir.dt.bfloat16
    Sig = mybir.ActivationFunctionType.Sigmoid
    Exp = mybir.ActivationFunctionType.Exp
    Tanh = mybir.ActivationFunctionType.Tanh
    Mult = mybir.AluOpType.mult
    AP = bass.AP

    nc = bass.Bass()

    q4_d = nc.dram_tensor("q4", [128, QPC], bf16, kind="ExternalInput")
    K4_d = nc.dram_tensor("K4", [128, NK4], bf16, kind="ExternalInput")
    W_d = nc.dram_tensor("W32g", [128, 256], bf16, kind="ExternalInput")
    # Rm: NR rotation matrices then maskl/maskh [128, 256]
    R_d = nc.dram_tensor("Rm", [128, 128 * NR + 256], bf16,
                         kind="ExternalInput")
    Xe_d = nc.dram_tensor("Xe", [NKEY, 132], bf16, kind="ExternalInput")
    ba_d = nc.dram_tensor("bat", [128, 2], f32, kind="ExternalInput")
    out_d = nc.dram_tensor("out", [QPC, 132], f32, kind="ExternalOutput")

    al = nc.alloc_sbuf_tensor
    q4 = al("q4s", [128, QPC], bf16)
    K4 = al("K4s", [128, NK4], bf16)
    W32 = al("W32s", [128, 256], bf16)
    Rm = al("Rms", [128, 128 * NR + 256], bf16)
    X5 = al("X5s", [128, 660], bf16)
    bat = al("bats", [128, 2], f32)
    arg = [al(f"arg{i}", [128, AMAX], bf16) for i in range(3)]
    sig = al("sigs", [128, QPC], bf16)
    ee = al("ees", [128, QPC], bf16)
    M0 = al("M0s", [128, QPC], bf16)
    El = [al(f"el{t}", [128, 128], bf16) for t in range(4)]
    Eh = [al(f"eh{t}", [128, 128], bf16) for t in range(4)]
    ov = [al(f"ov{t}", [128, 132], f32) for t in range(4)]
    MKL = 128 * NRC              # maskl col offset in Rm
    MKH = 128 * NRC + 128

    def rcol(r):
        i = RIDX[r]
        return 128 * i if i < NRC else 128 * i + 256

    ap_ = nc.alloc_psum_tensor
    psS = [ap_(f"psS{i}", [128, max(SIZES)], f32) for i in range(3)]
    psA = ap_("psA", [128, QPC], f32)
    psB = ap_("psB", [128, QPC], f32)
    vp = [ap_(f"vp{i}", [128, 132], f32) for i in range(2)]

    sem = nc.alloc_semaphore
    sINA, sINB, sINC, sIND, sINE, sINF, sING, sINH, sINR = (sem(n) for n in
        ("sINA", "sINB", "sINC", "sIND", "sINE", "sINF", "sING", "sINH",
         "sINR"))
    sADD, sTANH, sMM, sSIG, sEXP = (sem(n) for n in
        ("sADD", "sTANH", "sMM", "sSIG", "sEXP"))
    sSH, sSCP, sMSK, sMSKP, sVMM, sEPI, sOUT = (sem(n) for n in
        ("sSH", "sSCP", "sMSK", "sMSKP", "sVMM", "sEPI", "sOUT"))

    W0 = SIZES[0]
    K4P = 4 * 31 + W0 + 16       # first-chunk K4 cols (rounded up)

    with nc.Block() as block:

        @block.sync
        def _(sync):
            # SP queue: q4p, q4r, K4r, bat, Xe, Rm-extra
            sync.dma_start(q4[:, 0:W0], q4_d[:, 0:W0]).then_inc(sINA, 16)
            sync.dma_start(q4[:, W0:QPC], q4_d[:, W0:QPC]).then_inc(sINH, 16)
            sync.dma_start(K4[:, K4P:NK4], K4_d[:, K4P:NK4]).then_inc(sINC, 16)
            sync.dma_start(bat[:, :], ba_d[:, :]).then_inc(sIND, 16)
            sync.dma_start(AP(X5, 0, [[660, 128], [132, 5], [1, 132]]),
                           AP(Xe_d, 0, [[132, 128], [128 * 132, 5], [1, 132]])
                           ).then_inc(sINE, 16)
            sync.dma_start(Rm[:, 128 * NRC + 256:],
                           R_d[:, 128 * NRC + 256:]).then_inc(sINR, 16)
            for t in range(4):
                sync.wait_ge(sEPI, t + 1)
                sync.dma_start(out_d[128 * t:128 * (t + 1), :],
                               ov[t][:, :]).then_inc(sOUT, 16)
            sync.wait_ge(sOUT, 64)

        @block.vector
        def _(vector):
            def add(c):
                c0, W = C0S[c], SIZES[c]
                if c == 0:
                    vector.wait_ge(sINA, 16)
                    vector.wait_ge(sING, 16)
                elif c == 1:
                    vector.wait_ge(sINH, 16)
                    vector.wait_ge(sINC, 16)
                else:
                    vector.wait_ge(sMM, c - 2)       # arg slot c%3 free
                a = arg[c % 3]
                vector.tensor_add(
                    AP(a, 0, [[AMAX, 128], [W, U], [1, W]]),
                    AP(q4, c0, [[QPC, 128], [0, U], [1, W]]),
                    AP(K4, c0, [[NK4, 128], [4, U], [1, W]])
                ).then_inc(sADD, 1)

            def m0copy(c):
                c0, W = C0S[c], SIZES[c]
                vector.wait_ge(sSH, SHTHR[c] + 1)
                assert c < NCH - NDIR
                vector.tensor_copy(M0[:, c0:c0 + W],
                                   psA[:, c0:c0 + W]).then_inc(sSCP, 1)

            def el(c):
                vector.wait_ge(sSH, SHTHR[c + 1])
                for (t, a, b) in PIECES[c]:
                    la, lb = a - 128 * t, b - 128 * t
                    vector.tensor_tensor(
                        El[t][:, la:lb], psB[:, a:b],
                        Rm[:, MKL + la:MKL + lb], Mult).then_inc(sMSK, 1)
                    vector.tensor_tensor(
                        Eh[t][:, la:lb], psB[:, a:b],
                        Rm[:, MKH + la:MKH + lb], Mult).then_inc(sMSK, 1)

            def epi(t):
                vector.wait_ge(sVMM, t + 1)
                vector.tensor_copy(ov[t][:, :], vp[t % 2][:, :]
                                   ).then_inc(sEPI, 1)

            add(0); add(1); add(2); add(3); add(4)
            m0copy(0); el(0)
            add(5)
            m0copy(1); el(1)
            m0copy(2); el(2)
            epi(0)
            m0copy(3); el(3)
            epi(1)
            el(4)
            el(5)
            epi(2)
            epi(3)

        @block.scalar
        def _(scalar):
            def tanh(c):
                W = SIZES[c]
                scalar.wait_ge(sADD, c + 1)
                a = arg[c % 3]
                scalar.activation(AP(a, 0, [[AMAX, 128], [1, U * W]]),
                                  AP(a, 0, [[AMAX, 128], [1, U * W]]),
                                  Tanh).then_inc(sTANH, 1)

            def sigm(c):
                c0, W = C0S[c], SIZES[c]
                if c == 0:
                    scalar.wait_ge(sIND, 16)
                scalar.wait_ge(sMM, c + 1)
                scalar.activation(sig[:, c0:c0 + W], psS[c % 3][:, 0:W],
                                  Sig, bias=bat[:, 0:1]).then_inc(sSIG, 1)

            def expo(c):
                c0, W = C0S[c], SIZES[c]
                scalar.wait_ge(sSIG, c + 1)
                scalar.activation(ee[:, c0:c0 + W], sig[:, c0:c0 + W],
                                  Exp).then_inc(sEXP, 1)

            # sig/exp software-pipelined so the sSIG wait is pre-satisfied
            tanh(0); tanh(1)
            sigm(0)
            tanh(2)
            for c in range(1, NCH):
                sigm(c)
                expo(c - 1)
                if c + 2 < NCH:
                    tanh(c + 2)
            expo(NCH - 1)

        @block.tensor
        def _(tensor):
            def score(c):
                c0, W = C0S[c], SIZES[c]
                tensor.wait_ge(sTANH, c + 1)
                if c == 0:
                    tensor.wait_ge(sINB, 16)
                if c >= 3:
                    tensor.wait_ge(sSIG, c - 2)      # psS[c%3] free
                a = arg[c % 3]
                for ds in range(4):
                    for g in range(8):
                        mm = tensor.matmul(
                            psS[c % 3][32 * ds:32 * (ds + 1), 0:W],
                            W32[:, 32 * g:32 * (g + 1)],
                            AP(a, (8 * ds + g) * W, [[AMAX, 128], [1, W]]),
                            start=(g == 0), stop=(g == 7),
                            tile_position=(0, 32 * ds))
                        if ds == 3 and g == 7:
                            mm.then_inc(sMM, 1)

            def shear(c):
                c0, W = C0S[c], SIZES[c]
                nlo = W // 8
                # L-lo: rot lo on cols c0+lo::8, e -> psA (compacted)
                tensor.wait_ge(sEXP, c + 1)
                if c == 0:
                    tensor.wait_ge(sINF, 16)
                else:
                    # L-lo group start wipes psA bank: prior m0copy must
                    # be done; same for psB vs prior chunk's El/Eh reads
                    tensor.wait_ge(sSCP, c)
                    tensor.wait_ge(sMSK, MSKCUM[c - 1])
                for lo in range(8):
                    r = rcol(lo)
                    mm = tensor.matmul(
                        psA[:, c0 + lo * nlo:c0 + (lo + 1) * nlo],
                        Rm[:, r:r + 128],
                        AP(ee, c0 + lo, [[QPC, 128], [8, nlo]]),
                        start=(lo == 0), stop=(lo == 7),
                        skip_group_check=True)
                    if lo == 7:
                        mm.then_inc(sSH, 1)
                # L-hi: rot 8*((c0/8 + hi)%16) on stride-nlo classes,
                # M0 -> psB; compaction digits recombine to natural order
                tensor.wait_ge(sSCP, c + 1)
                for hi in range(nlo):
                    r = rcol(8 * ((c0 // 8 + hi) % 16))
                    mm = tensor.matmul(
                        psB[:, c0 + 8 * hi:c0 + 8 * (hi + 1)],
                        Rm[:, r:r + 128],
                        AP(M0, c0 + hi, [[QPC, 128], [nlo, 8]]),
                        start=(hi == 0), stop=(hi == nlo - 1),
                        skip_group_check=True)
                    if hi == nlo - 1:
                        mm.then_inc(sSH, 1)

            def shear_direct(c):
                c0, W = C0S[c], SIZES[c]
                tensor.wait_ge(sEXP, c + 1)
                if c == NCH - NDIR:
                    tensor.wait_ge(sINR, 16)
                tensor.wait_ge(sMSK, MSKCUM[c - 1])
                for il in range(W):
                    r = rcol((c0 + il) % 128)
                    mm = tensor.matmul(
                        psB[:, c0 + il:c0 + il + 1],
                        Rm[:, r:r + 128],
                        ee[:, c0 + il:c0 + il + 1],
                        start=(il == 0), stop=(il == W - 1),
                        skip_group_check=True)
                    if il == W - 1:
                        mm.then_inc(sSH, 1)

            def value(t):
                tensor.wait_ge(sMSK, MSKTHR[t])
                if t == 0:
                    tensor.wait_ge(sINE, 16)
                if t >= 2:
                    tensor.wait_ge(sEPI, t - 1)      # vp slot t%2 free
                tensor.matmul(vp[t % 2][:, :], El[t][:, :],
                              X5[:, 132 * t:132 * t + 132],
                              start=True, stop=False, skip_group_check=True)
                tensor.matmul(vp[t % 2][:, :], Eh[t][:, :],
                              X5[:, 132 * (t + 1):132 * (t + 1) + 132],
                              start=False, stop=True,
                              skip_group_check=True).then_inc(sVMM, 1)

            for c in range(NCH):
                score(c)
                if c >= 2:
                    shear(c - 2)
                for t in range(4):
                    if LASTCH[t] == c - 3:
                        value(t)
            shear_direct(NCH - 2)
            for t in range(4):
                if LASTCH[t] == NCH - 3:
                    value(t)
            shear_direct(NCH - 1)
            for t in range(4):
                if LASTCH[t] >= NCH - 2:
                    value(t)

        @block.gpsimd
        def _(gpsimd):
            gpsimd.dma_start(K4[:, 0:K4P], K4_d[:, 0:K4P]).then_inc(sING, 16)
            gpsimd.dma_start(W32[:, :], W_d[:, :]).then_inc(sINB, 16)
            gpsimd.dma_start(Rm[:, 0:128 * NRC + 256],
                             R_d[:, 0:128 * NRC + 256]).then_inc(sINF, 16)

    nc.finalize()
    return nc


def _prep_inputs(x, mask, Wt, Wx, bh, Wa, ba):
    """Build the 8 per-core input maps (host-side sharding + projections)."""
    x64 = x.astype(np.float64)
    # rotation matrices R_r[(c - r) % 128, c] = 1, then tri masks
    Rm = np.zeros((128, 128 * NR + 256), np.float32)
    cix = np.arange(128)
    for i, r in enumerate(ROTS):
        base = 128 * i if i < NRC else 128 * i + 256
        Rm[(cix - r) % 128, base + cix] = 1.0
    cc = cix[:, None]
    il = cix[None, :]
    Rm[:, 128 * NRC:128 * NRC + 128] = (cc >= il).astype(np.float32)
    Rm[:, 128 * NRC + 128:128 * NRC + 256] = (cc < il).astype(np.float32)
    Rm = Rm.astype(BF16)
    # W32g: mm g maps rows (dm,u) -> col 4g+dm with weight Wa[u]
    W32 = np.zeros((128, 256), np.float32)
    for g in range(8):
        for dm in range(4):
            W32[32 * dm:32 * (dm + 1), 32 * g + 4 * g + dm] = Wa[:, 0]
    W32 = W32.astype(BF16)
    in_maps = []
    for c in range(NCORES):
        b = c // 4
        qs = (c % 4) * QPC
        q = (x64[b] @ Wt.astype(np.float64) + bh.astype(np.float64))
        k = (x64[b] @ Wx.astype(np.float64))
        qT = q[qs:qs + QPC].T.astype(np.float32)          # [32, 512]
        q4 = np.tile(qT, (4, 1)).astype(BF16)             # [128, 512]
        kx = np.zeros((NKEY + 8, U), np.float64)
        lo = qs - 64
        s0, s1 = max(0, lo), min(L, lo + NKEY)
        kx[s0 - lo:s1 - lo] = k[s0:s1]
        K4 = np.zeros((128, NK4), np.float32)
        for dm in range(4):
            K4[32 * dm:32 * (dm + 1), :] = kx[dm:dm + NK4].T
        K4 = K4.astype(BF16)
        Xe = np.zeros((NKEY, 132), np.float32)
        mk = mask[b].astype(np.float32)
        xr = np.zeros((NKEY, F), np.float32)
        xr[s0 - lo:s1 - lo] = x[b, s0:s1] * mk[s0:s1, None]
        Xe[:, :F] = xr
        val = np.zeros(NKEY, np.float32)
        val[s0 - lo:s1 - lo] = mk[s0:s1]
        Xe[:, F] = val
        Xe = Xe.astype(BF16)
        bat = np.zeros((128, 2), np.float32)
        bat[:, 0] = float(ba[0])
        in_maps.append({"q4": q4, "K4": K4, "W32g": W32, "Rm": Rm,
                        "Xe": Xe, "bat": bat})
    return in_maps


def kernel(x, mask, Wt, Wx, bh, Wa, ba, _want_results=False):
    global _built
    from concourse.bass_utils import run_bass_kernel_spmd
    x = np.asarray(x)
    mask = np.asarray(mask)
    Wt, Wx, bh, Wa, ba = (np.asarray(a) for a in (Wt, Wx, bh, Wa, ba))
    if _built is None:
        _built = _build()
    nc = _built
    in_maps = _prep_inputs(x, mask, Wt, Wx, bh, Wa, ba)
    res = run_bass_kernel_spmd(nc, in_maps, core_ids=list(range(NCORES)))
    v = np.zeros((B, L, F), np.float32)
    for c in range(NCORES):
        b = c // 4
        qs = (c % 4) * QPC
        o = res.results[c]["out"]
        v[b, qs:qs + QPC] = o[:, :F] / (o[:, F:F + 1] + EPS)
    v *= mask.astype(np.float32)[:, :, None]
    if _want_results:
        return v, res
    return v


# revision 24
# speedup vs baseline: 1.1602x; 1.0642x over previous
"""Banded additive attention (width-128) on 8 TRN2 NeuronCores — raw Bass.

Problem: B=2, L=2048, F=128, U=32, WIDTH=128
  q = x@Wt + bh, k = x@Wx                       (host, like baseline)
  s_ij = Wa . tanh(q_i + k_j)                   (j in [i-64, i+63])
  e_ij = exp(sigmoid(s_ij)) * band * mask
  v_i  = sum_j e_ij x_j / (sum_j e_ij + 1e-7)

Sharding: core c handles batch c//4, queries [(c%4)*512, +512).  No
collectives.  Raw Bass; standalone wait_ge with hand-computed thresholds.

v4 pipeline (query-chunked; chunk widths SIZES, all mult of 16):
  DVE  : add   arg[p=(dm,u), (dg,il)] = q4[p, il] + K4[p, 4*dg + il]
         (3 arg slots so adds run 2 chunks ahead)
  ACT  : tanh  in-place on arg (the bulk: 512*128*32 elems/core)
  PE   : 32 accumulating matmuls per chunk into psS[c%3] psum bank:
         mm(ds,g): lhsT = W32g[:, 32g:+32] (block-diag Wa), rhs = dg-block,
         out rows 32ds..+32 (tile_position col = 32ds), d = 32ds+4g+dm
  ACT  : sig = Sigmoid(psum + ba); e = Exp(sig)
  PE   : 2-level shear (radix 8x16): rotation matmuls on col classes with
         compacted outputs (digits recombine to natural order):
         L-lo: rot lo=il%8 on stride-8 classes, e -> psA
         L-hi: rot 8*((c0/8 + hi)%16) on stride-(W/8) classes, M0 -> psB
         (last chunk W=32: single level, 32 one-col rotation matmuls)
  DVE  : M0 = copy(psA);  El = psB * lower-tri mask
  Pool : Eh = psB * upper-tri mask  (parallel with El)
  PE   : v = El.T @ X[t] + Eh.T @ X[t+1]  (X has validity col -> denom)
  DVE  : ov = vp;  DMA out.  Host divides by denom and applies mask.
"""

import numpy as np
import ml_dtypes

B, L, F, U = 2, 2048, 128, 32
WIDTH = 128
EPS = 1e-7
NCORES = 8
QPC = (B * L) // NCORES          # 512 queries per core
NKEY = QPC + WIDTH               # 640 key rows per core
NK4 = NKEY + 4                   # K4 row pitch
BF16 = ml_dtypes.bfloat16

SIZES = [80, 112, 128, 128, 48, 16]      # query chunks (mult of 16, <=128)
C0S = [0, 80, 192, 320, 448, 496]
NCH = len(SIZES)
NDIR = 4                                 # last NDIR chunks use direct shear
# core rotations (2-level shear) + direct rotations for the last chunks
ROTS = [0, 1, 2, 3, 4, 5, 6, 7] + [8 * k for k in range(1, 16)]
NRC = len(ROTS)                          # 23 core matrices
DROTS = []
for c in range(NCH - NDIR, NCH):
    DROTS += [(C0S[c] + il) % 128 for il in range(SIZES[c])]
for r in DROTS:
    if r not in ROTS:
        ROTS.append(r)
RIDX = {r: i for i, r in enumerate(ROTS)}
NR = len(ROTS)
AMAX = max(SIZES) * U            # 4608: arg buffer cols

# (block, global lo, global hi) pieces per chunk; block t = cols [128t,+128)
PIECES = []
for c in range(NCH):
    lo, hi = C0S[c], C0S[c] + SIZES[c]
    ps = []
    for t in range(4):
        a, b = max(lo, 128 * t), min(hi, 128 * (t + 1))
        if a < b:
            ps.append((t, a, b))
    PIECES.append(ps)
LASTCH = {}                      # chunk completing block t's El/Eh
for c in range(NCH):
    for (t, a, b) in PIECES[c]:
        LASTCH[t] = c
_msk = 0
MSKTHR = {}                      # sMSK threshold per block (2 per piece)
MSKCUM = []                      # cumulative TT count through chunk c
for c in range(NCH):
    for (t, a, b) in PIECES[c]:
        _msk += 2
        MSKTHR[t] = _msk
    MSKCUM.append(_msk)
# sSH increments: 2 per 2-level chunk, 1 per direct chunk.
# SHTHR[c] = count before chunk c's shear; SHTHR[NCH] = total
SHTHR = [0]
for c in range(NCH):
    SHTHR.append(SHTHR[-1] + (1 if c >= NCH - NDIR else 2))
assert max(SIZES) <= 128 and all(w % 16 == 0 for w in SIZES)

_built = None


def _build():
    import concourse.bass as bass
    import concourse.mybir as mybir

    f32 = mybir.dt.float32
    bf16 = mybir.dt.bfloat16
    Sig = mybir.ActivationFunctionType.Sigmoid
    Exp = mybir.ActivationFunctionType.Exp
    Tanh = mybir.ActivationFunctionType.Tanh
    Mult = mybir.AluOpType.mult
    AP = bass.AP

    nc = bass.Bass()

    q4_d = nc.dram_tensor("q4", [128, QPC], bf16, kind="ExternalInput")
    K4_d = nc.dram_tensor("K4", [128, NK4], bf16, kind="ExternalInput")
    W_d = nc.dram_tensor("W32g", [128, 256], bf16, kind="ExternalInput")
    # Rm: NR rotation matrices then maskl/maskh [128, 256]
    R_d = nc.dram_tensor("Rm", [128, 128 * NR + 256], bf16,
                         kind="ExternalInput")
    Xe_d = nc.dram_tensor("Xe", [NKEY, 132], bf16, kind="ExternalInput")
    ba_d = nc.dram_tensor("bat", [128, 2], f32, kind="ExternalInput")
    out_d = nc.dram_tensor("out", [QPC, 132], f32, kind="ExternalOutput")

    al = nc.alloc_sbuf_tensor
    q4 = al("q4s", [128, QPC], bf16)
    K4 = al("K4s", [128, NK4], bf16)
    W32 = al("W32s", [128, 256], bf16)
    Rm = al("Rms", [128, 128 * NR + 256], bf16)
    X5 = al("X5s", [128, 660], bf16)
    bat = al("bats", [128, 2], f32)
    arg = [al(f"arg{i}", [128, AMAX], bf16) for i in range(3)]
    sig = al("sigs", [128, QPC], bf16)
    ee = al("ees", [128, QPC], bf16)
    M0 = al("M0s", [128, QPC], bf16)
    El = [al(f"el{t}", [128, 128], bf16) for t in range(4)]
    Eh = [al(f"eh{t}", [128, 128], bf16) for t in range(4)]
    ov = [al(f"ov{t}", [128, 132], f32) for t in range(4)]
    MKL = 128 * NRC              # maskl col offset in Rm
    MKH = 128 * NRC + 128

    def rcol(r):
        i = RIDX[r]
        return 128 * i if i < NRC else 128 * i + 256

    ap_ = nc.alloc_psum_tensor
    psS = [ap_(f"psS{i}", [128, max(SIZES)], f32) for i in range(3)]
    psA = ap_("psA", [128, QPC], f32)
    psB = ap_("psB", [128, QPC], f32)
    vp = [ap_(f"vp{i}", [128, 132], f32) for i in range(2)]

    sem = nc.alloc_semaphore
    sINA, sINB, sINC, sIND, sINE, sINF, sING, sINH, sINR = (sem(n) for n in
        ("sINA", "sINB", "sINC", "sIND", "sINE", "sINF", "sING", "sINH",
         "sINR"))
    sADD, sTANH, sMM, sSIG, sEXP = (sem(n) for n in
        ("sADD", "sTANH", "sMM", "sSIG", "sEXP"))
    sSH, sSCP, sMSK, sMSKP, sVMM, sEPI, sOUT = (sem(n) for n in
        ("sSH", "sSCP", "sMSK", "sMSKP", "sVMM", "sEPI", "sOUT"))

    W0 = SIZES[0]
    K4P = 4 * 31 + W0 + 16       # first-chunk K4 cols (rounded up)

    with nc.Block() as block:

        @block.sync
        def _(sync):
            # SP queue: K4p, K4r, bat, Xe, Rm-extra (pool gets q4/W32/Rm)
            sync.dma_start(K4[:, 0:K4P], K4_d[:, 0:K4P]).then_inc(sINA, 16)
            sync.dma_start(K4[:, K4P:NK4], K4_d[:, K4P:NK4]).then_inc(sINC, 16)
            sync.dma_start(bat[:, :], ba_d[:, :]).then_inc(sIND, 16)
            sync.dma_start(AP(X5, 0, [[660, 128], [132, 5], [1, 132]]),
                           AP(Xe_d, 0, [[132, 128], [128 * 132, 5], [1, 132]])
                           ).then_inc(sINE, 16)
            sync.dma_start(Rm[:, 128 * NRC + 256:],
                           R_d[:, 128 * NRC + 256:]).then_inc(sINR, 16)
            for t in range(4):
                sync.wait_ge(sEPI, t + 1)
                sync.dma_start(out_d[128 * t:128 * (t + 1), :],
                               ov[t][:, :]).then_inc(sOUT, 16)
            sync.wait_ge(sOUT, 64)

        @block.vector
        def _(vector):
            def add(c):
                c0, W = C0S[c], SIZES[c]
                if c == 0:
                    vector.wait_ge(sINA, 16)
                    vector.wait_ge(sING, 16)
                elif c == 1:
                    vector.wait_ge(sINH, 16)
                    vector.wait_ge(sINC, 16)
                else:
                    vector.wait_ge(sMM, c - 2)       # arg slot c%3 free
                a = arg[c % 3]
                vector.tensor_add(
                    AP(a, 0, [[AMAX, 128], [W, U], [1, W]]),
                    AP(q4, c0, [[QPC, 128], [0, U], [1, W]]),
                    AP(K4, c0, [[NK4, 128], [4, U], [1, W]])
                ).then_inc(sADD, 1)

            def m0copy(c):
                c0, W = C0S[c], SIZES[c]
                vector.wait_ge(sSH, SHTHR[c] + 1)
                assert c < NCH - NDIR
                vector.tensor_copy(M0[:, c0:c0 + W],
                                   psA[:, c0:c0 + W]).then_inc(sSCP, 1)

            def el(c):
                vector.wait_ge(sSH, SHTHR[c + 1])
                for (t, a, b) in PIECES[c]:
                    la, lb = a - 128 * t, b - 128 * t
                    if c >= NCH - NDIR and c != NCH - NDIR:
                        srcT, off = psS[c % 3], a - C0S[c]
                    else:
                        srcT, off = psB, a
                    vector.tensor_tensor(
                        El[t][:, la:lb], srcT[:, off:off + (b - a)],
                        Rm[:, MKL + la:MKL + lb], Mult).then_inc(sMSK, 1)
                    vector.tensor_tensor(
                        Eh[t][:, la:lb], srcT[:, off:off + (b - a)],
                        Rm[:, MKH + la:MKH + lb], Mult).then_inc(sMSK, 1)

            def epi(t):
                vector.wait_ge(sVMM, t + 1)
                vector.tensor_copy(ov[t][:, :], vp[t % 2][:, :]
                                   ).then_inc(sEPI, 1)

            add(0); add(1); add(2); add(3); add(4)
            m0copy(0); el(0)
            add(5)
            m0copy(1); el(1)
            epi(0)
            el(2)
            epi(1)
            el(3)
            el(4)
            el(5)
            epi(2)
            epi(3)

        @block.scalar
        def _(scalar):
            def tanh(c):
                W = SIZES[c]
                scalar.wait_ge(sADD, c + 1)
                a = arg[c % 3]
                scalar.activation(AP(a, 0, [[AMAX, 128], [1, U * W]]),
                                  AP(a, 0, [[AMAX, 128], [1, U * W]]),
                                  Tanh).then_inc(sTANH, 1)

            def sigm(c):
                c0, W = C0S[c], SIZES[c]
                if c == 0:
                    scalar.wait_ge(sIND, 16)
                scalar.wait_ge(sMM, c + 1)
                scalar.activation(sig[:, c0:c0 + W], psS[c % 3][:, 0:W],
                                  Sig, bias=bat[:, 0:1]).then_inc(sSIG, 1)

            def expo(c):
                c0, W = C0S[c], SIZES[c]
                scalar.wait_ge(sSIG, c + 1)
                scalar.activation(ee[:, c0:c0 + W], sig[:, c0:c0 + W],
                                  Exp).then_inc(sEXP, 1)

            # sig/exp software-pipelined so the sSIG wait is pre-satisfied
            tanh(0); tanh(1)
            sigm(0)
            tanh(2)
            for c in range(1, NCH):
                sigm(c)
                expo(c - 1)
                if c + 2 < NCH:
                    tanh(c + 2)
            expo(NCH - 1)

        @block.tensor
        def _(tensor):
            def score(c):
                c0, W = C0S[c], SIZES[c]
                tensor.wait_ge(sTANH, c + 1)
                if c == 0:
                    tensor.wait_ge(sINB, 16)
                if c >= 3:
                    tensor.wait_ge(sSIG, c - 2)      # psS[c%3] free
                a = arg[c % 3]
                for ds in range(4):
                    for g in range(8):
                        mm = tensor.matmul(
                            psS[c % 3][32 * ds:32 * (ds + 1), 0:W],
                            W32[:, 32 * g:32 * (g + 1)],
                            AP(a, (8 * ds + g) * W, [[AMAX, 128], [1, W]]),
                            start=(g == 0), stop=(g == 7),
                            tile_position=(0, 32 * ds))
                        if ds == 3 and g == 7:
                            mm.then_inc(sMM, 1)

            def llo(c):
                c0, W = C0S[c], SIZES[c]
                nlo = W // 8
                # L-lo: rot lo on cols c0+lo::8, e -> psA (compacted)
                tensor.wait_ge(sEXP, c + 1)
                if c == 0:
                    tensor.wait_ge(sINF, 16)
                else:
                    # group start wipes the psA bank: prior m0copy first
                    tensor.wait_ge(sSCP, c)
                for lo in range(8):
                    r = rcol(lo)
                    mm = tensor.matmul(
                        psA[:, c0 + lo * nlo:c0 + (lo + 1) * nlo],
                        Rm[:, r:r + 128],
                        AP(ee, c0 + lo, [[QPC, 128], [8, nlo]]),
                        start=(lo == 0), stop=(lo == 7),
                        skip_group_check=True)
                    if lo == 7:
                        mm.then_inc(sSH, 1)

            def lhi(c):
                c0, W = C0S[c], SIZES[c]
                nlo = W // 8
                # L-hi: rot 8*((c0/8 + hi)%16) on stride-nlo classes,
                # M0 -> psB; compaction digits recombine to natural order
                tensor.wait_ge(sSCP, c + 1)
                if c >= 1:
                    # group start wipes psB: prior chunk's El/Eh reads first
                    tensor.wait_ge(sMSK, MSKCUM[c - 1])
                for hi in range(nlo):
                    r = rcol(8 * ((c0 // 8 + hi) % 16))
                    mm = tensor.matmul(
                        psB[:, c0 + 8 * hi:c0 + 8 * (hi + 1)],
                        Rm[:, r:r + 128],
                        AP(M0, c0 + hi, [[QPC, 128], [nlo, 8]]),
                        start=(hi == 0), stop=(hi == nlo - 1),
                        skip_group_check=True)
                    if hi == nlo - 1:
                        mm.then_inc(sSH, 1)

            def shear_direct(c):
                c0, W = C0S[c], SIZES[c]
                tensor.wait_ge(sEXP, c + 1)
                if c == NCH - NDIR:
                    # first direct chunk -> psB (free after prior El/Eh)
                    tensor.wait_ge(sINR, 16)
                    tensor.wait_ge(sMSK, MSKCUM[c - 1])
                    dstT, base = psB, c0
                else:
                    # -> the chunk's own score bank (free after its sig)
                    dstT, base = psS[c % 3], 0
                for il in range(W):
                    r = rcol((c0 + il) % 128)
                    mm = tensor.matmul(
                        dstT[:, base + il:base + il + 1],
                        Rm[:, r:r + 128],
                        ee[:, c0 + il:c0 + il + 1],
                        start=(il == 0), stop=(il == W - 1),
                        skip_group_check=True)
                    if il == W - 1:
                        mm.then_inc(sSH, 1)

            def value(t):
                tensor.wait_ge(sMSK, MSKTHR[t])
                if t == 0:
                    tensor.wait_ge(sINE, 16)
                if t >= 2:
                    tensor.wait_ge(sEPI, t - 1)      # vp slot t%2 free
                tensor.matmul(vp[t % 2][:, :], El[t][:, :],
                              X5[:, 132 * t:132 * t + 132],
                              start=True, stop=False, skip_group_check=True)
                tensor.matmul(vp[t % 2][:, :], Eh[t][:, :],
                              X5[:, 132 * (t + 1):132 * (t + 1) + 132],
                              start=False, stop=True,
                              skip_group_check=True).then_inc(sVMM, 1)

            NL = NCH - NDIR                  # chunks with 2-level shear
            for c in range(NCH):
                if c == NCH - 1:
                    # fill PE idle before the last score: first direct chunk
                    shear_direct(NL)
                score(c)
                if 3 <= c and c - 3 < NL:
                    lhi(c - 3)
                if 2 <= c and c - 2 < NL:
                    llo(c - 2)
                for t in range(4):
                    if LASTCH[t] == c - 3:
                        value(t)
            if NL - 1 > NCH - 4:             # lhi not yet emitted in-loop
                lhi(NL - 1)
            for c in range(NL + 1, NCH):
                shear_direct(c)
            for t in range(4):
                if LASTCH[t] > NCH - 4:
                    value(t)

        @block.gpsimd
        def _(gpsimd):
            gpsimd.dma_start(q4[:, 0:W0], q4_d[:, 0:W0]).then_inc(sING, 16)
            gpsimd.dma_start(q4[:, W0:QPC], q4_d[:, W0:QPC]).then_inc(sINH, 16)
            gpsimd.dma_start(W32[:, :], W_d[:, :]).then_inc(sINB, 16)
            gpsimd.dma_start(Rm[:, 0:128 * NRC + 256],
                             R_d[:, 0:128 * NRC + 256]).then_inc(sINF, 16)

    nc.finalize()
    return nc


def _prep_inputs(x, mask, Wt, Wx, bh, Wa, ba):
    """Build the 8 per-core input maps (host-side sharding + projections)."""
    x64 = x.astype(np.float64)
    # rotation matrices R_r[(c - r) % 128, c] = 1, then tri masks
    Rm = np.zeros((128, 128 * NR + 256), np.float32)
    cix = np.arange(128)
    for i, r in enumerate(ROTS):
        base = 128 * i if i < NRC else 128 * i + 256
        Rm[(cix - r) % 128, base + cix] = 1.0
    cc = cix[:, None]
    il = cix[None, :]
    Rm[:, 128 * NRC:128 * NRC + 128] = (cc >= il).astype(np.float32)
    Rm[:, 128 * NRC + 128:128 * NRC + 256] = (cc < il).astype(np.float32)
    Rm = Rm.astype(BF16)
    # W32g: mm g maps rows (dm,u) -> col 4g+dm with weight Wa[u]
    W32 = np.zeros((128, 256), np.float32)
    for g in range(8):
        for dm in range(4):
            W32[32 * dm:32 * (dm + 1), 32 * g + 4 * g + dm] = Wa[:, 0]
    W32 = W32.astype(BF16)
    in_maps = []
    for c in range(NCORES):
        b = c // 4
        qs = (c % 4) * QPC
        q = (x64[b] @ Wt.astype(np.float64) + bh.astype(np.float64))
        k = (x64[b] @ Wx.astype(np.float64))
        qT = q[qs:qs + QPC].T.astype(np.float32)          # [32, 512]
        q4 = np.tile(qT, (4, 1)).astype(BF16)             # [128, 512]
        kx = np.zeros((NKEY + 8, U), np.float64)
        lo = qs - 64
        s0, s1 = max(0, lo), min(L, lo + NKEY)
        kx[s0 - lo:s1 - lo] = k[s0:s1]
        K4 = np.zeros((128, NK4), np.float32)
        for dm in range(4):
            K4[32 * dm:32 * (dm + 1), :] = kx[dm:dm + NK4].T
        K4 = K4.astype(BF16)
        Xe = np.zeros((NKEY, 132), np.float32)
        mk = mask[b].astype(np.float32)
        xr = np.zeros((NKEY, F), np.float32)
        xr[s0 - lo:s1 - lo] = x[b, s0:s1] * mk[s0:s1, None]
        Xe[:, :F] = xr
        val = np.zeros(NKEY, np.float32)
        val[s0 - lo:s1 - lo] = mk[s0:s1]
        Xe[:, F] = val
        Xe = Xe.astype(BF16)
        bat = np.zeros((128, 2), np.float32)
        bat[:, 0] = float(ba[0])
        in_maps.append({"q4": q4, "K4": K4, "W32g": W32, "Rm": Rm,
                        "Xe": Xe, "bat": bat})
    return in_maps


def kernel(x, mask, Wt, Wx, bh, Wa, ba, _want_results=False):
    global _built
    from concourse.bass_utils import run_bass_kernel_spmd
    x = np.asarray(x)
    mask = np.asarray(mask)
    Wt, Wx, bh, Wa, ba = (np.asarray(a) for a in (Wt, Wx, bh, Wa, ba))
    if _built is None:
        _built = _build()
    nc = _built
    in_maps = _prep_inputs(x, mask, Wt, Wx, bh, Wa, ba)
    res = run_bass_kernel_spmd(nc, in_maps, core_ids=list(range(NCORES)))
    v = np.zeros((B, L, F), np.float32)
    for c in range(NCORES):
        b = c // 4
        qs = (c % 4) * QPC
        o = res.results[c]["out"]
        v[b, qs:qs + QPC] = o[:, :F] / (o[:, F:F + 1] + EPS)
    v *= mask.astype(np.float32)[:, :, None]
    if _want_results:
        return v, res
    return v
